# revision 1
# baseline (speedup 1.0000x reference)
"""Trainium2 Bass kernel for nn_LinearDiffusion (truncated Taylor expm(a) @ x).

Math: a = row-normalized symmetric scatter of per-head edge weights onto an
(H, N, N) zero tensor; result = sum_{i=0..6} a^i x / i! with x = h reshaped
per-head.

Strategy (8 NeuronCores, one chip):
  * The adjacency is ~0.4% dense; the dense einsum would stream 1 GB of
    matrix 6x. Instead: sparse formulation with the pattern preprocessed on
    host into per-core tables.
  * Node features of all 4 heads are kept together: one node row = 64 fp32
    = 256 B, the exact granularity of `dma_gather`.
  * Shard by destination row: core k owns rows [k*1024, (k+1)*1024).
    Edge entries (r, c, w) sorted by r, padded into 128-edge chunks that
    each scatter into one 128-row block.
  * Per iteration, per core:
      1. dma_gather of x[src] rows (256 B each) from a DRAM copy of x
      2. VectorE: weighted product, split hi/lo fp16 (exact to ~2^-22)
      3. TensorE: per chunk, one-hot scatter matrix (fp8, SBUF-resident)
         x [hi|lo] rhs -> accumulate the block's (128, 128) PSUM tile
      4. evacuate PSUM, accumulate Taylor term, AllGather new x
  * Only the table *data* differs per core, so one SPMD program serves all
    8 cores; per-core tables arrive as inputs.
"""

import math
from dataclasses import dataclass

import numpy as np

import concourse.bass as bass  # noqa: F401  (kept for callers)
import concourse.tile as tile
from concourse import bacc, mybir
from concourse.bass_utils import run_bass_kernel_spmd

# ----------------------------------------------------------------- config

N, H, E, D = 8192, 4, 131072, 64
d = D // H
NCORES = 8
BLK = 128  # dst-block size == PE stationary width
K_TAYLOR = 6


@dataclass(frozen=True)
class Cfg:
    n: int = N
    n_cores: int = NCORES
    hi_lo_split: bool = True  # False -> single fp16 product (faster, ~5e-4 err)

    @property
    def rows_per_core(self):
        return self.n // self.n_cores

    @property
    def blocks_per_core(self):
        return self.rows_per_core // BLK


# ----------------------------------------------------------- preprocessing


def _entries(e, src, dst, n):
    """Unique symmetric entries with 'last write wins' duplicate semantics,
    matching jax's .at[].set() on CPU. Returns (rows, cols, w[H, nnz])."""
    src = src.astype(np.int64)
    dst = dst.astype(np.int64)
    n_edges = len(src)
    keys = np.concatenate([src * n + dst, dst * n + src])
    eid = np.concatenate([np.arange(n_edges), np.arange(n_edges)])
    order = np.arange(2 * n_edges)
    perm = np.lexsort((-order, keys))
    k_sorted = keys[perm]
    first = np.ones(len(k_sorted), dtype=bool)
    first[1:] = k_sorted[1:] != k_sorted[:-1]
    win = perm[first]
    ukeys = k_sorted[first]
    rows = (ukeys // n).astype(np.int64)
    cols = (ukeys % n).astype(np.int64)
    weids = eid[win]
    vals = e[:, weids].astype(np.float64)  # (H, nnz)
    nheads = e.shape[0]
    rowsum = np.zeros((nheads, n), dtype=np.float64)
    for hh in range(nheads):
        rowsum[hh] = np.bincount(rows, weights=vals[hh], minlength=n)
    w = (vals / rowsum[:, rows]).astype(np.float32)
    return rows, cols, w


def _make_tables(e, src, dst, cfg: Cfg):
    """Per-core device tables. Returns (tables, nch) where tables is a list
    over cores of dicts with keys idx (int16), w4 (fp32), sca (fp8)."""
    import ml_dtypes

    n = cfg.n
    rows, cols, w = _entries(e, src, dst, n)
    nheads = w.shape[0]
    bpc = cfg.blocks_per_core

    order = np.argsort(rows, kind="stable")
    rows_s, cols_s, w_s = rows[order], cols[order], w[:, order]
    blk = rows_s // BLK
    nblocks = n // BLK
    starts = np.searchsorted(blk, np.arange(nblocks + 1))
    bcnt = np.diff(starts)
    bmax = int(np.ceil(bcnt.max() / 128))  # chunks per block, uniform
    nch = bpc * bmax

    tables = []
    for k in range(cfg.n_cores):
        idx = np.zeros((nch, 128), dtype=np.int16)
        w4 = np.zeros((128, nch, nheads), dtype=np.float32)
        sca = np.zeros((128, nch, 128), dtype=ml_dtypes.float8_e4m3fn)
        for j in range(bpc):
            b = k * bpc + j
            s, cnt = starts[b], bcnt[b]
            sl = slice(s, s + cnt)
            eloc = np.arange(cnt)
            c_local = j * bmax + eloc // 128
            p = eloc % 128
            idx[c_local, p] = cols_s[sl].astype(np.int16)
            w4[p, c_local, :] = w_s[:, sl].T
            m = rows_s[sl] - b * BLK
            sca[p, c_local, m] = 1.0
        # dma_gather index layout: logical index i -> [i % 16, i // 16],
        # replicated across the 8 groups of 16 partitions.
        seq = idx.reshape(-1)  # logical order: i = c*128 + p
        wrapped = seq.reshape(-1, 16).T  # (16, nch*8)
        idx_t = np.tile(wrapped, (8, 1))  # (128, nch*8)
        tables.append(
            {
                "idx": np.ascontiguousarray(idx_t),
                "w4": np.ascontiguousarray(w4.reshape(128, nch * nheads)),
                "sca": np.ascontiguousarray(sca.reshape(128, nch * 128)),
            }
        )
    return tables, nch


# ------------------------------------------------------------ bass program

_FP32 = mybir.dt.float32
_FP16 = mybir.dt.float16
_FP8 = mybir.dt.float8e4
_I16 = mybir.dt.int16


def _build_program(cfg: Cfg, nch: int):
    n = cfg.n
    bpc = cfg.blocks_per_core
    bmax = nch // bpc
    rpc = cfg.rows_per_core
    nc = bacc.Bacc(
        "TRN2",
        target_bir_lowering=False,
        debug=False,
        num_devices=cfg.n_cores,
    )

    xin = nc.dram_tensor("xin", [n, D], _FP32, kind="ExternalInput").ap()
    x0s_d = nc.dram_tensor("x0s", [rpc, D], _FP32, kind="ExternalInput").ap()
    idx_d = nc.dram_tensor("idx", [128, nch * 8], _I16, kind="ExternalInput").ap()
    w4_d = nc.dram_tensor("w4", [128, nch * H], _FP32, kind="ExternalInput").ap()
    sca_d = nc.dram_tensor("sca", [128, nch * 128], _FP8, kind="ExternalInput").ap()
    out_d = nc.dram_tensor("out", [rpc, D], _FP32, kind="ExternalOutput").ap()

    xall = nc.dram_tensor("xall", [n, D], _FP32, addr_space="Shared").ap()
    slice_in = nc.dram_tensor("slice_in", [rpc, D], _FP32).ap()

    groups = [list(range(cfg.n_cores))]

    # Sub-batch the per-iteration work so each dma_gather stays under the
    # SWDGE descriptor-ring capacity (~9k indices per call observed safe).
    halves = 1
    while nch // halves * 128 > 9216 or bpc % halves:
        halves += 1
        assert halves <= bpc, "cannot find sub-batch split"
    hbpc = bpc // halves  # blocks per sub-batch
    hch = nch // halves  # chunks per sub-batch

    with tile.TileContext(nc) as tc:
        with (
            tc.tile_pool(name="tables", bufs=1) as tp,
            tc.tile_pool(name="xg", bufs=2) as xgp,
            tc.tile_pool(name="xgw", bufs=2) as xgwp,
            tc.tile_pool(name="acc", bufs=1) as accp,
            tc.tile_pool(name="stage", bufs=2) as stp,
            tc.tile_pool(name="psum", bufs=4, space="PSUM") as pp,
        ):
            idx_sb = tp.tile([128, nch * 8], _I16)
            w4_sb = tp.tile([128, nch, H], _FP32)
            sca_sb = tp.tile([128, nch * 128], _FP8)
            nc.sync.dma_start(out=idx_sb[:], in_=idx_d)
            nc.sync.dma_start(
                out=w4_sb[:].rearrange("p c h -> p (c h)"), in_=w4_d
            )
            nc.sync.dma_start(out=sca_sb[:], in_=sca_d)

            # x0: full copy into the gather buffer + this core's slice into
            # the running Taylor accumulator (identity term).
            nc.sync.dma_start(out=xall, in_=xin)
            result = accp.tile([128, bpc, D], _FP32)
            nc.sync.dma_start(
                out=result[:],
                in_=x0s_d.rearrange("(j p) f -> p j f", p=128),
            )

            for it in range(1, K_TAYLOR + 1):
                coef = 1.0 / math.factorial(it)
                xnext = stp.tile([128, bpc, D], _FP32, tag="xnext")
                for hf in range(halves):
                    c0 = hf * hch
                    xg = xgp.tile([128, hch, D], _FP32, tag="xg")
                    nc.gpsimd.dma_gather(
                        xg[:],
                        xall,
                        idx_sb[:, c0 * 8 : (c0 + hch) * 8],
                        hch * 128,
                        hch * 128,
                        D,
                        single_packet=False,
                    )
                    # prod = xg * w4 (broadcast each head weight over d)
                    xg4 = xg[:].rearrange("p c (h f) -> p c h f", h=H)
                    w4v = (
                        w4_sb[:, c0 : c0 + hch, :]
                        .unsqueeze(3)
                        .to_broadcast([128, hch, H, d])
                    )
                    xgw = xgwp.tile([128, hch, 2 * D], _FP16, tag="xgw")
                    hi = xgw[:, :, 0:D].rearrange("p c (h f) -> p c h f", h=H)
                    lo = xgw[:, :, D : 2 * D].rearrange(
                        "p c (h f) -> p c h f", h=H
                    )
                    if cfg.hi_lo_split:
                        nc.vector.tensor_mul(xg4, xg4, w4v)
                        nc.scalar.copy(hi, xg4)
                        nc.vector.tensor_sub(lo, xg4, hi)
                    else:
                        nc.vector.tensor_mul(hi, xg4, w4v)
                        nc.vector.memset(xgw[:, :, D : 2 * D], 0.0)

                    for jj in range(hf * hbpc, (hf + 1) * hbpc):
                        ps = pp.tile([128, 2 * D], _FP32, tag="ps")
                        for b in range(bmax):
                            c = jj * bmax + b
                            nc.tensor.matmul(
                                ps[:],
                                lhsT=sca_sb[:, c * 128 : (c + 1) * 128],
                                rhs=xgw[:, c - c0, :],
                                start=(b == 0),
                                stop=(b == bmax - 1),
                            )
                        nc.scalar.copy(xnext[:, jj, :], ps[:, 0:D])
                        nc.vector.tensor_add(
                            xnext[:, jj, :], xnext[:, jj, :], ps[:, D : 2 * D]
                        )
                        nc.vector.scalar_tensor_tensor(
                            result[:, jj, :],
                            xnext[:, jj, :],
                            coef,
                            result[:, jj, :],
                            op0=mybir.AluOpType.mult,
                            op1=mybir.AluOpType.add,
                        )
                if it < K_TAYLOR:
                    nc.sync.dma_start(
                        out=slice_in.rearrange("(j p) f -> p j f", p=128),
                        in_=xnext[:],
                    )
                    nc.gpsimd.collective_compute(
                        "AllGather",
                        mybir.AluOpType.bypass,
                        replica_groups=groups,
                        ins=[slice_in],
                        outs=[xall],
                    )

            nc.sync.dma_start(
                out=out_d.rearrange("(j p) f -> p j f", p=128),
                in_=result[:],
            )

    nc.compile()
    return nc


# ------------------------------------------------------------------ driver

_CACHE = {}


def _get_program(cfg: Cfg, nch: int):
    key = (cfg, nch)
    if key not in _CACHE:
        _CACHE[key] = _build_program(cfg, nch)
    return _CACHE[key]


def _in_maps(x0, tables, cfg: Cfg):
    rpc = cfg.rows_per_core
    return [
        {
            "xin": x0,
            "x0s": np.ascontiguousarray(x0[k * rpc : (k + 1) * rpc]),
            "idx": t["idx"],
            "w4": t["w4"],
            "sca": t["sca"],
        }
        for k, t in enumerate(tables)
    ]


def run(h, e, src, dst, cfg: Cfg = Cfg(), trace: bool = False):
    """Full pipeline: preprocess, build/compile (cached), execute, assemble."""
    h = np.asarray(h, dtype=np.float32)
    e = np.asarray(e, dtype=np.float32)
    src = np.asarray(src)
    dst = np.asarray(dst)
    nheads = e.shape[0]
    n = h.shape[0]
    dd = h.shape[1] // nheads
    assert (n, nheads, dd) == (cfg.n, H, d), (n, nheads, dd)

    tables, nch = _make_tables(e, src, dst, cfg)
    x0 = np.ascontiguousarray(
        h.reshape(nheads, n, dd).transpose(1, 0, 2).reshape(n, nheads * dd)
    )
    nc = _get_program(cfg, nch)
    res = run_bass_kernel_spmd(
        nc,
        _in_maps(x0, tables, cfg),
        list(range(cfg.n_cores)),
        trace=trace,
    )
    out = np.concatenate(
        [res.results[k]["out"] for k in range(cfg.n_cores)], axis=0
    )
    # back to reference layout: (n, H, d) node-major -> (H, n, d) -> (N, D)
    out = np.ascontiguousarray(out.reshape(n, nheads, dd).transpose(1, 0, 2)).reshape(
        n, nheads * dd
    )
    return out, res


def kernel(h, e, src, dst):
    out, _ = run(h, e, src, dst)
    return out



# revision 3
# speedup vs baseline: 2.4855x; 2.4855x over previous
"""Trainium2 Bass kernel for nn_LinearDiffusion (truncated Taylor expm(a) @ x).

Math: a = row-normalized symmetric scatter of per-head edge weights onto an
(H, N, N) zero tensor; result = sum_{i=0..6} a^i x / i! with x = h reshaped
per-head.

Strategy (8 NeuronCores, one chip):
  * Sparse formulation, dst-row sharded: core k owns rows [k*1024,(k+1)*1024).
  * x (8192 x 64, all heads interleaved per node row) lives in SBUF as fp16
    (1 MB); no DMA row-gather at all.  Both the gather x[src] and the
    scatter into dst rows run on the Tensor engine as one-hot fp8 matmuls:
      - per (dst-block I, src-block J) pair a fixed 32*R-slot region
        (R = ceil(max pair edge count / 32), 3 for this instance);
        gather: psG[slots] = G_pair^T @ x_J  (col-tiled, 32-aligned pieces)
      - VectorE: per-head weight multiply, PSUM -> fp16 SBUF
      - scatter: psY_I += S_chunk^T @ xgw_chunk  (fp8 one-hot stationary)
  * The slot layout (pairs -> 96-slot regions -> 128-slot chunks) is
    identical on every core, so one SPMD program serves all 8 cores; only
    the table *data* (one-hot columns, weights) differs per core.
  * Between the k=6 matmuls: fp16 AllGather of the new x slices.
"""

import math
from dataclasses import dataclass

import numpy as np

import concourse.bass as bass  # noqa: F401  (kept for callers)
import concourse.tile as tile
from concourse import bacc, mybir
from concourse.bass_utils import run_bass_kernel_spmd

# ----------------------------------------------------------------- config

N, H, E, D = 8192, 4, 131072, 64
d = D // H
NCORES = 8
BLK = 128  # dst/src block size == PE width
NB = N // BLK  # 64 src blocks
K_TAYLOR = 6


@dataclass(frozen=True)
class Cfg:
    n: int = N
    n_cores: int = NCORES
    hi_lo_split: bool = True  # unused (fp16 path is exact enough); kept for test.py

    @property
    def rows_per_core(self):
        return self.n // self.n_cores

    @property
    def blocks_per_core(self):
        return self.rows_per_core // BLK


# ----------------------------------------------------------- preprocessing


def _entries(e, src, dst, n):
    """Unique symmetric entries with 'last write wins' duplicate semantics,
    matching jax's .at[].set() on CPU. Returns (rows, cols, w[H, nnz])."""
    src = src.astype(np.int64)
    dst = dst.astype(np.int64)
    n_edges = len(src)
    keys = np.concatenate([src * n + dst, dst * n + src])
    eid = np.concatenate([np.arange(n_edges), np.arange(n_edges)])
    order = np.arange(2 * n_edges)
    perm = np.lexsort((-order, keys))
    k_sorted = keys[perm]
    first = np.ones(len(k_sorted), dtype=bool)
    first[1:] = k_sorted[1:] != k_sorted[:-1]
    win = perm[first]
    ukeys = k_sorted[first]
    rows = (ukeys // n).astype(np.int64)
    cols = (ukeys % n).astype(np.int64)
    weids = eid[win]
    vals = e[:, weids].astype(np.float64)  # (H, nnz)
    nheads = e.shape[0]
    rowsum = np.zeros((nheads, n), dtype=np.float64)
    for hh in range(nheads):
        rowsum[hh] = np.bincount(rows, weights=vals[hh], minlength=n)
    w = (vals / rowsum[:, rows]).astype(np.float32)
    return rows, cols, w


def _pieces_for_ps(ps):
    """Decompose each src-block pair's [j*ps, (j+1)*ps) slot range of a dst
    block into PE col-tile-legal (chunk, offset, width, j) pieces.
    Legal (offset, width): widths are multiples of 32 with offset+width<=128
    and offset in {0,32,64,96}; at offset 32 only width 32, at 64 width<=64,
    at 96 width 32, at 0 any (asserts in bass.matmul tile_position)."""
    wmax = {0: 128, 32: 32, 64: 64, 96: 32}
    cb = (NB * ps) // 128  # chunks per dst block
    pieces = [[] for _ in range(cb)]
    for j in range(NB):
        s, rem = j * ps, ps
        while rem:
            c, a = s // 128, s % 128
            wd = min(rem, wmax[a], 128 - a)
            pieces[c].append((a, wd, j, s))
            s += wd
            rem -= wd
    assert sum(len(p) for p in pieces) <= 2 * NB + cb
    return pieces, cb


def _make_tables(e, src, dst, cfg: Cfg):
    """Per-core device tables. Returns (tables, R) where tables is a list
    over cores of dicts with keys gsl, sca (fp8), w4 (fp32)."""
    import ml_dtypes

    n = cfg.n
    rows, cols, w = _entries(e, src, dst, n)
    nheads = w.shape[0]
    bpc = cfg.blocks_per_core

    # global R so the program structure is identical on every core
    cnt = np.zeros((n // BLK, NB), dtype=np.int64)
    np.add.at(cnt, (rows // BLK, cols // BLK), 1)
    R = max(3, int(np.ceil(cnt.max() / 32)))
    ps = 32 * R
    cb = (NB * ps) // 128  # chunks per dst block
    nch = bpc * cb
    nslots = nch * 128

    tables = []
    for k in range(cfg.n_cores):
        sel = (rows >= k * 1024) & (rows < (k + 1) * 1024)
        r_k, c_k, w_k = rows[sel], cols[sel], w[:, sel]
        b_loc = r_k // BLK - k * bpc  # 0..7
        j_blk = c_k // BLK
        key = b_loc * NB + j_blk
        order = np.argsort(key, kind="stable")
        r_k, c_k, w_k, key = r_k[order], c_k[order], w_k[:, order], key[order]
        grp_start = np.searchsorted(key, key)  # first index of each group
        off_in_pair = np.arange(len(key)) - grp_start
        assert off_in_pair.max(initial=0) < ps
        slot = key * ps + off_in_pair  # key = b_loc * NB + j_blk

        gsl = np.zeros((128, nslots), dtype=ml_dtypes.float8_e4m3fn)
        sca = np.zeros((128, nslots), dtype=ml_dtypes.float8_e4m3fn)
        w4 = np.zeros((128, nch, nheads), dtype=np.float32)
        gsl[c_k % BLK, slot] = 1.0
        sca[slot % 128, (slot // 128) * 128 + (r_k % BLK)] = 1.0
        w4[slot % 128, slot // 128, :] = w_k.T
        tables.append(
            {
                "gsl": gsl,
                "sca": sca,
                "w4": np.ascontiguousarray(w4.reshape(128, nch * nheads)),
            }
        )
    return tables, R


# ------------------------------------------------------------ bass program

_FP32 = mybir.dt.float32
_FP16 = mybir.dt.float16
_FP8 = mybir.dt.float8e4


def _build_program(cfg: Cfg, R: int):
    n = cfg.n
    bpc = cfg.blocks_per_core
    rpc = cfg.rows_per_core
    ps = 32 * R
    pieces, cb = _pieces_for_ps(ps)
    nch = bpc * cb
    nslots = nch * 128
    assert cb % 8 == 0
    nwaves = cb // 8

    nc = bacc.Bacc(
        "TRN2",
        target_bir_lowering=False,
        debug=False,
        num_devices=cfg.n_cores,
    )

    xin16 = nc.dram_tensor("xin16", [n, D], _FP16, kind="ExternalInput").ap()
    x0s_d = nc.dram_tensor("x0s", [rpc, D], _FP32, kind="ExternalInput").ap()
    gsl_d = nc.dram_tensor("gsl", [128, nslots], _FP8, kind="ExternalInput").ap()
    sca_d = nc.dram_tensor("sca", [128, nslots], _FP8, kind="ExternalInput").ap()
    w4_d = nc.dram_tensor("w4", [128, nch * H], _FP32, kind="ExternalInput").ap()
    out_d = nc.dram_tensor("out", [rpc, D], _FP32, kind="ExternalOutput").ap()

    xall = nc.dram_tensor("xall", [n, D], _FP16, addr_space="Shared").ap()
    slice_in = nc.dram_tensor("slice_in", [rpc, D], _FP16).ap()

    groups = [list(range(cfg.n_cores))]

    with tile.TileContext(nc) as tc:
        with (
            tc.tile_pool(name="tables", bufs=1) as tp,
            tc.tile_pool(name="xsb", bufs=2) as xsbp,
            tc.tile_pool(name="xgw", bufs=3) as xgwp,
            tc.tile_pool(name="acc", bufs=1) as accp,
            tc.tile_pool(name="stage", bufs=2) as stp,
            tc.tile_pool(name="psg", bufs=3, space="PSUM") as ppg,
            tc.tile_pool(name="psy", bufs=2, space="PSUM") as ppy,
        ):
            gsl_sb = tp.tile([128, nslots], _FP8)
            sca_sb = tp.tile([128, nslots], _FP8)
            w4_sb = tp.tile([128, nch, H], _FP32)
            nc.sync.dma_start(out=gsl_sb[:], in_=gsl_d)
            nc.sync.dma_start(out=sca_sb[:], in_=sca_d)
            nc.sync.dma_start(out=w4_sb[:].rearrange("p c h -> p (c h)"), in_=w4_d)

            # Taylor accumulator starts at the identity term (this core's x0).
            result = accp.tile([128, bpc, D], _FP32)
            nc.sync.dma_start(
                out=result[:],
                in_=x0s_d.rearrange("(j p) f -> p j f", p=128),
            )

            for it in range(1, K_TAYLOR + 1):
                coef = 1.0 / math.factorial(it)
                src_ap = (xin16 if it == 1 else xall).rearrange(
                    "(b p) f -> p b f", p=128
                )
                xsb = xsbp.tile([128, NB, D], _FP16, tag="xsb")
                for g in range(4):
                    nc.sync.dma_start(
                        out=xsb[:, g * 16 : (g + 1) * 16, :],
                        in_=src_ap[:, g * 16 : (g + 1) * 16, :],
                    )

                xnext = stp.tile([128, bpc, D], _FP16, tag="xnext")
                for b in range(bpc):
                    psY = ppy.tile([128, D], _FP32, tag="psY")
                    for wv in range(nwaves):
                        psG = ppg.tile([128, 8, D], _FP32, tag="psG")
                        xgw = xgwp.tile([128, 8, D], _FP16, tag="xgw")
                        for ci in range(8):
                            c = wv * 8 + ci
                            for (a, wd, j, s_blk) in pieces[c]:
                                gofs = b * NB * ps + s_blk
                                nc.tensor.matmul(
                                    psG[a : a + wd, ci, :],
                                    lhsT=gsl_sb[:, gofs : gofs + wd],
                                    rhs=xsb[:, j, :],
                                    start=True,
                                    stop=True,
                                    tile_position=(0, a),
                                )
                        gc0 = b * cb + wv * 8
                        psG4 = psG[:].rearrange("p c (h f) -> p c h f", h=H)
                        xgw4 = xgw[:].rearrange("p c (h f) -> p c h f", h=H)
                        w4v = (
                            w4_sb[:, gc0 : gc0 + 8, :]
                            .unsqueeze(3)
                            .to_broadcast([128, 8, H, d])
                        )
                        nc.vector.tensor_mul(xgw4, psG4, w4v)
                        for ci in range(8):
                            c = wv * 8 + ci
                            gc = b * cb + c
                            nc.tensor.matmul(
                                psY[:],
                                lhsT=sca_sb[:, gc * 128 : (gc + 1) * 128],
                                rhs=xgw[:, ci, :],
                                start=(c == 0),
                                stop=(c == cb - 1),
                            )
                    nc.scalar.copy(xnext[:, b, :], psY[:])
                    nc.vector.scalar_tensor_tensor(
                        result[:, b, :],
                        psY[:],
                        coef,
                        result[:, b, :],
                        op0=mybir.AluOpType.mult,
                        op1=mybir.AluOpType.add,
                    )
                if it < K_TAYLOR:
                    nc.sync.dma_start(
                        out=slice_in.rearrange("(j p) f -> p j f", p=128),
                        in_=xnext[:],
                    )
                    nc.gpsimd.collective_compute(
                        "AllGather",
                        mybir.AluOpType.bypass,
                        replica_groups=groups,
                        ins=[slice_in],
                        outs=[xall],
                    )

            nc.sync.dma_start(
                out=out_d.rearrange("(j p) f -> p j f", p=128),
                in_=result[:],
            )

    nc.compile()
    return nc


# ------------------------------------------------------------------ driver

_CACHE = {}


def _get_program(cfg: Cfg, R: int):
    key = (cfg, R)
    if key not in _CACHE:
        _CACHE[key] = _build_program(cfg, R)
    return _CACHE[key]


def _in_maps(x0, tables, cfg: Cfg):
    rpc = cfg.rows_per_core
    x16 = x0.astype(np.float16)
    return [
        {
            "xin16": x16,
            "x0s": np.ascontiguousarray(x0[k * rpc : (k + 1) * rpc]),
            "gsl": t["gsl"],
            "sca": t["sca"],
            "w4": t["w4"],
        }
        for k, t in enumerate(tables)
    ]


def run(h, e, src, dst, cfg: Cfg = Cfg(), trace: bool = False):
    """Full pipeline: preprocess, build/compile (cached), execute, assemble."""
    h = np.asarray(h, dtype=np.float32)
    e = np.asarray(e, dtype=np.float32)
    src = np.asarray(src)
    dst = np.asarray(dst)
    nheads = e.shape[0]
    n = h.shape[0]
    dd = h.shape[1] // nheads
    assert (n, nheads, dd) == (cfg.n, H, d), (n, nheads, dd)

    tables, R = _make_tables(e, src, dst, cfg)
    x0 = np.ascontiguousarray(
        h.reshape(nheads, n, dd).transpose(1, 0, 2).reshape(n, nheads * dd)
    )
    nc = _get_program(cfg, R)
    res = run_bass_kernel_spmd(
        nc,
        _in_maps(x0, tables, cfg),
        list(range(cfg.n_cores)),
        trace=trace,
    )
    out = np.concatenate(
        [res.results[k]["out"] for k in range(cfg.n_cores)], axis=0
    )
    # back to reference layout: (n, H, d) node-major -> (H, n, d) -> (N, D)
    out = np.ascontiguousarray(out.reshape(n, nheads, dd).transpose(1, 0, 2)).reshape(
        n, nheads * dd
    )
    return out, res


def kernel(h, e, src, dst):
    out, _ = run(h, e, src, dst)
    return out


# revision 4
# speedup vs baseline: 5.5630x; 2.2382x over previous
"""Trainium2 Bass kernel for nn_LinearDiffusion (truncated Taylor expm(a) @ x).

Math: a = row-normalized symmetric scatter of per-head edge weights onto an
(H, N, N) zero tensor; result = sum_{i=0..6} a^i x / i! with x = h reshaped
per-head.

Strategy (8 NeuronCores, one chip):
  * Sparse formulation, dst-row sharded: core k owns rows [k*1024,(k+1)*1024).
  * x (8192 x 64, all heads interleaved per node row) lives in SBUF as fp16
    (1 MB); no DMA row-gather at all.  Both the gather x[src] and the
    scatter into dst rows run on the Tensor engine as one-hot fp8 matmuls:
      - per (dst-block I, src-block J) pair a fixed 32*R-slot region
        (R = ceil(max pair edge count / 32), 3 for this instance);
        gather: psG[slots] = G_pair^T @ x_J  (col-tiled, 32-aligned pieces)
      - VectorE: per-head weight multiply, PSUM -> fp16 SBUF
      - scatter: psY_I += S_chunk^T @ xgw_chunk  (fp8 one-hot stationary)
  * The slot layout (pairs -> 96-slot regions -> 128-slot chunks) is
    identical on every core, so one SPMD program serves all 8 cores; only
    the table *data* (one-hot columns, weights) differs per core.
  * Between the k=6 matmuls: fp16 AllGather of the new x slices.
"""

import math
from dataclasses import dataclass

import numpy as np

import concourse.bass as bass  # noqa: F401  (kept for callers)
import concourse.tile as tile
from concourse import bacc, mybir
from concourse.bass_utils import run_bass_kernel_spmd

# ----------------------------------------------------------------- config

N, H, E, D = 8192, 4, 131072, 64
d = D // H
NCORES = 8
BLK = 128  # dst/src block size == PE width
NB = N // BLK  # 64 src blocks
K_TAYLOR = 6


@dataclass(frozen=True)
class Cfg:
    n: int = N
    n_cores: int = NCORES
    hi_lo_split: bool = True  # unused (fp16 path is exact enough); kept for test.py

    @property
    def rows_per_core(self):
        return self.n // self.n_cores

    @property
    def blocks_per_core(self):
        return self.rows_per_core // BLK


# ----------------------------------------------------------- preprocessing


def _entries(e, src, dst, n):
    """Unique symmetric entries with 'last write wins' duplicate semantics,
    matching jax's .at[].set() on CPU. Returns (rows, cols, w[H, nnz])."""
    src = src.astype(np.int64)
    dst = dst.astype(np.int64)
    n_edges = len(src)
    keys = np.concatenate([src * n + dst, dst * n + src])
    eid = np.concatenate([np.arange(n_edges), np.arange(n_edges)])
    order = np.arange(2 * n_edges)
    perm = np.lexsort((-order, keys))
    k_sorted = keys[perm]
    first = np.ones(len(k_sorted), dtype=bool)
    first[1:] = k_sorted[1:] != k_sorted[:-1]
    win = perm[first]
    ukeys = k_sorted[first]
    rows = (ukeys // n).astype(np.int64)
    cols = (ukeys % n).astype(np.int64)
    weids = eid[win]
    vals = e[:, weids].astype(np.float64)  # (H, nnz)
    nheads = e.shape[0]
    rowsum = np.zeros((nheads, n), dtype=np.float64)
    for hh in range(nheads):
        rowsum[hh] = np.bincount(rows, weights=vals[hh], minlength=n)
    w = (vals / rowsum[:, rows]).astype(np.float32)
    return rows, cols, w


def _pieces_for_ps(ps):
    """Decompose each src-block pair's [j*ps, (j+1)*ps) slot range of a dst
    block into PE col-tile-legal (chunk, offset, width, j) pieces.
    Legal (offset, width): widths are multiples of 32 with offset+width<=128
    and offset in {0,32,64,96}; at offset 32 only width 32, at 64 width<=64,
    at 96 width 32, at 0 any (asserts in bass.matmul tile_position)."""
    wmax = {0: 128, 32: 32, 64: 64, 96: 32}
    cb = (NB * ps) // 128  # chunks per dst block
    pieces = [[] for _ in range(cb)]
    for j in range(NB):
        s, rem = j * ps, ps
        while rem:
            c, a = s // 128, s % 128
            wd = min(rem, wmax[a], 128 - a)
            pieces[c].append((a, wd, j, s))
            s += wd
            rem -= wd
    assert sum(len(p) for p in pieces) <= 2 * NB + cb
    return pieces, cb


def _make_tables(e, src, dst, cfg: Cfg):
    """Per-core device tables. Returns (tables, R) where tables is a list
    over cores of dicts with keys gsl, sca (fp8), w4 (fp32)."""
    import ml_dtypes

    n = cfg.n
    rows, cols, w = _entries(e, src, dst, n)
    nheads = w.shape[0]
    bpc = cfg.blocks_per_core

    # global R so the program structure is identical on every core.
    # R is a multiple of 4 => every pair occupies whole 128-slot chunks, so
    # every gather matmul loads a full 128-col stationary (FWL-eligible, no
    # column tiling, no PE array mode switches).
    cnt = np.zeros((n // BLK, NB), dtype=np.int64)
    np.add.at(cnt, (rows // BLK, cols // BLK), 1)
    R = 4 * int(np.ceil(cnt.max() / 128))
    ps = 32 * R
    cb = (NB * ps) // 128  # chunks per dst block
    nch = bpc * cb
    nslots = nch * 128

    tables = []
    for k in range(cfg.n_cores):
        sel = (rows >= k * 1024) & (rows < (k + 1) * 1024)
        r_k, c_k, w_k = rows[sel], cols[sel], w[:, sel]
        b_loc = r_k // BLK - k * bpc  # 0..7
        j_blk = c_k // BLK
        key = b_loc * NB + j_blk
        order = np.argsort(key, kind="stable")
        r_k, c_k, w_k, key = r_k[order], c_k[order], w_k[:, order], key[order]
        grp_start = np.searchsorted(key, key)  # first index of each group
        off_in_pair = np.arange(len(key)) - grp_start
        assert off_in_pair.max(initial=0) < ps
        slot = key * ps + off_in_pair  # key = b_loc * NB + j_blk

        gsl = np.zeros((128, nslots), dtype=ml_dtypes.float8_e4m3fn)
        sca = np.zeros((128, nslots), dtype=ml_dtypes.float8_e4m3fn)
        w4 = np.zeros((128, nch, nheads), dtype=np.float32)
        gsl[c_k % BLK, slot] = 1.0
        sca[slot % 128, (slot // 128) * 128 + (r_k % BLK)] = 1.0
        w4[slot % 128, slot // 128, :] = w_k.T
        tables.append(
            {
                "gsl": gsl,
                "sca": sca,
                "w4": np.ascontiguousarray(w4.reshape(128, nch * nheads)),
            }
        )
    return tables, R


# ------------------------------------------------------------ bass program

_FP32 = mybir.dt.float32
_FP16 = mybir.dt.float16
_FP8 = mybir.dt.float8e4


def _build_program(cfg: Cfg, R: int):
    n = cfg.n
    bpc = cfg.blocks_per_core
    rpc = cfg.rows_per_core
    ps = 32 * R
    pieces, cb = _pieces_for_ps(ps)
    nch = bpc * cb
    nslots = nch * 128
    assert cb % 8 == 0
    nwaves = cb // 8

    nc = bacc.Bacc(
        "TRN2",
        target_bir_lowering=False,
        debug=False,
        num_devices=cfg.n_cores,
    )

    xin16 = nc.dram_tensor("xin16", [n, D], _FP16, kind="ExternalInput").ap()
    x0s_d = nc.dram_tensor("x0s", [rpc, D], _FP32, kind="ExternalInput").ap()
    gsl_d = nc.dram_tensor("gsl", [128, nslots], _FP8, kind="ExternalInput").ap()
    sca_d = nc.dram_tensor("sca", [128, nslots], _FP8, kind="ExternalInput").ap()
    w4_d = nc.dram_tensor("w4", [128, nch * H], _FP32, kind="ExternalInput").ap()
    out_d = nc.dram_tensor("out", [rpc, D], _FP32, kind="ExternalOutput").ap()

    xall = nc.dram_tensor("xall", [n, D], _FP16, addr_space="Shared").ap()
    slice_in = nc.dram_tensor("slice_in", [rpc, D], _FP16).ap()

    groups = [list(range(cfg.n_cores))]

    with tile.TileContext(nc) as tc:
        with (
            tc.tile_pool(name="tables", bufs=1) as tp,
            tc.tile_pool(name="xsb", bufs=2) as xsbp,
            tc.tile_pool(name="xgw", bufs=3) as xgwp,
            tc.tile_pool(name="acc", bufs=1) as accp,
            tc.tile_pool(name="stage", bufs=2) as stp,
            tc.tile_pool(name="psg", bufs=3, space="PSUM") as ppg,
            tc.tile_pool(name="psy", bufs=2, space="PSUM") as ppy,
        ):
            gsl_sb = tp.tile([128, nslots], _FP8)
            sca_sb = tp.tile([128, nslots], _FP8)
            w4_sb = tp.tile([128, nch, H], _FP32)
            nc.sync.dma_start(out=gsl_sb[:], in_=gsl_d)
            nc.sync.dma_start(out=sca_sb[:], in_=sca_d)
            nc.sync.dma_start(out=w4_sb[:].rearrange("p c h -> p (c h)"), in_=w4_d)

            # Taylor accumulator starts at the identity term (this core's x0).
            result = accp.tile([128, bpc, D], _FP32)
            nc.sync.dma_start(
                out=result[:],
                in_=x0s_d.rearrange("(j p) f -> p j f", p=128),
            )

            for it in range(1, K_TAYLOR + 1):
                coef = 1.0 / math.factorial(it)
                src_ap = (xin16 if it == 1 else xall).rearrange(
                    "(b p) f -> p b f", p=128
                )
                xsb = xsbp.tile([128, NB, D], _FP16, tag="xsb")
                for g in range(4):
                    nc.sync.dma_start(
                        out=xsb[:, g * 16 : (g + 1) * 16, :],
                        in_=src_ap[:, g * 16 : (g + 1) * 16, :],
                    )

                xnext = stp.tile([128, bpc, D], _FP16, tag="xnext")
                for b in range(bpc):
                    psY = ppy.tile([128, D], _FP32, tag="psY")
                    for wv in range(nwaves):
                        psG = ppg.tile([128, 8, D], _FP32, tag="psG")
                        xgw = xgwp.tile([128, 8, D], _FP16, tag="xgw")
                        for ci in range(8):
                            c = wv * 8 + ci
                            for (a, wd, j, s_blk) in pieces[c]:
                                gofs = b * NB * ps + s_blk
                                nc.tensor.matmul(
                                    psG[a : a + wd, ci, :],
                                    lhsT=gsl_sb[:, gofs : gofs + wd],
                                    rhs=xsb[:, j, :],
                                    start=True,
                                    stop=True,
                                    tile_position=(0, a),
                                )
                        gc0 = b * cb + wv * 8
                        psG4 = psG[:].rearrange("p c (h f) -> p c h f", h=H)
                        xgw4 = xgw[:].rearrange("p c (h f) -> p c h f", h=H)
                        w4v = (
                            w4_sb[:, gc0 : gc0 + 8, :]
                            .unsqueeze(3)
                            .to_broadcast([128, 8, H, d])
                        )
                        nc.vector.tensor_mul(xgw4, psG4, w4v)
                        for ci in range(8):
                            c = wv * 8 + ci
                            gc = b * cb + c
                            nc.tensor.matmul(
                                psY[:],
                                lhsT=sca_sb[:, gc * 128 : (gc + 1) * 128],
                                rhs=xgw[:, ci, :],
                                start=(c == 0),
                                stop=(c == cb - 1),
                            )
                    nc.scalar.copy(xnext[:, b, :], psY[:])
                    nc.vector.scalar_tensor_tensor(
                        result[:, b, :],
                        psY[:],
                        coef,
                        result[:, b, :],
                        op0=mybir.AluOpType.mult,
                        op1=mybir.AluOpType.add,
                    )
                if it < K_TAYLOR:
                    nc.sync.dma_start(
                        out=slice_in.rearrange("(j p) f -> p j f", p=128),
                        in_=xnext[:],
                    )
                    nc.gpsimd.collective_compute(
                        "AllGather",
                        mybir.AluOpType.bypass,
                        replica_groups=groups,
                        ins=[slice_in],
                        outs=[xall],
                    )

            nc.sync.dma_start(
                out=out_d.rearrange("(j p) f -> p j f", p=128),
                in_=result[:],
            )

    nc.compile()
    return nc


# ------------------------------------------------------------------ driver

_CACHE = {}


def _get_program(cfg: Cfg, R: int):
    key = (cfg, R)
    if key not in _CACHE:
        _CACHE[key] = _build_program(cfg, R)
    return _CACHE[key]


def _in_maps(x0, tables, cfg: Cfg):
    rpc = cfg.rows_per_core
    x16 = x0.astype(np.float16)
    return [
        {
            "xin16": x16,
            "x0s": np.ascontiguousarray(x0[k * rpc : (k + 1) * rpc]),
            "gsl": t["gsl"],
            "sca": t["sca"],
            "w4": t["w4"],
        }
        for k, t in enumerate(tables)
    ]


def run(h, e, src, dst, cfg: Cfg = Cfg(), trace: bool = False):
    """Full pipeline: preprocess, build/compile (cached), execute, assemble."""
    h = np.asarray(h, dtype=np.float32)
    e = np.asarray(e, dtype=np.float32)
    src = np.asarray(src)
    dst = np.asarray(dst)
    nheads = e.shape[0]
    n = h.shape[0]
    dd = h.shape[1] // nheads
    assert (n, nheads, dd) == (cfg.n, H, d), (n, nheads, dd)

    tables, R = _make_tables(e, src, dst, cfg)
    x0 = np.ascontiguousarray(
        h.reshape(nheads, n, dd).transpose(1, 0, 2).reshape(n, nheads * dd)
    )
    nc = _get_program(cfg, R)
    res = run_bass_kernel_spmd(
        nc,
        _in_maps(x0, tables, cfg),
        list(range(cfg.n_cores)),
        trace=trace,
    )
    out = np.concatenate(
        [res.results[k]["out"] for k in range(cfg.n_cores)], axis=0
    )
    # back to reference layout: (n, H, d) node-major -> (H, n, d) -> (N, D)
    out = np.ascontiguousarray(out.reshape(n, nheads, dd).transpose(1, 0, 2)).reshape(
        n, nheads * dd
    )
    return out, res


def kernel(h, e, src, dst):
    out, _ = run(h, e, src, dst)
    return out


# revision 10
# speedup vs baseline: 5.8656x; 1.0544x over previous
"""Trainium2 Bass kernel for nn_LinearDiffusion (truncated Taylor expm(a) @ x).

Math: a = row-normalized symmetric scatter of per-head edge weights onto an
(H, N, N) zero tensor; result = sum_{i=0..6} a^i x / i! with x = h reshaped
per-head.

Strategy (8 NeuronCores, one chip):
  * Sparse formulation, dst-row sharded: core k owns rows [k*1024,(k+1)*1024).
  * x (8192 x 64, all heads interleaved per node row) lives in SBUF as fp16
    (1 MB); no DMA row-gather at all.  Both the gather x[src] and the
    scatter into dst rows run on the Tensor engine as one-hot fp8 matmuls:
      - per (dst-block I, src-block J) pair a fixed 32*R-slot region
        (R = ceil(max pair edge count / 32), 3 for this instance);
        gather: psG[slots] = G_pair^T @ x_J  (col-tiled, 32-aligned pieces)
      - VectorE: per-head weight multiply, PSUM -> fp16 SBUF
      - scatter: psY_I += S_chunk^T @ xgw_chunk  (fp8 one-hot stationary)
  * The slot layout (pairs -> 96-slot regions -> 128-slot chunks) is
    identical on every core, so one SPMD program serves all 8 cores; only
    the table *data* (one-hot columns, weights) differs per core.
  * Between the k=6 matmuls: fp16 AllGather of the new x slices.
"""

import math
from dataclasses import dataclass

import numpy as np

import concourse.bass as bass  # noqa: F401  (kept for callers)
import concourse.tile as tile
from concourse import bacc, mybir
from concourse.bass_utils import run_bass_kernel_spmd

# ----------------------------------------------------------------- config

N, H, E, D = 8192, 4, 131072, 64
d = D // H
NCORES = 8
BLK = 128  # dst/src block size == PE width
NB = N // BLK  # 64 src blocks
K_TAYLOR = 6


@dataclass(frozen=True)
class Cfg:
    n: int = N
    n_cores: int = NCORES
    hi_lo_split: bool = True  # unused (fp16 path is exact enough); kept for test.py

    @property
    def rows_per_core(self):
        return self.n // self.n_cores

    @property
    def blocks_per_core(self):
        return self.rows_per_core // BLK


# ----------------------------------------------------------- preprocessing


def _entries(e, src, dst, n):
    """Unique symmetric entries with 'last write wins' duplicate semantics,
    matching jax's .at[].set() on CPU. Returns (rows, cols, w[H, nnz])."""
    src = src.astype(np.int64)
    dst = dst.astype(np.int64)
    n_edges = len(src)
    keys = np.concatenate([src * n + dst, dst * n + src])
    eid = np.concatenate([np.arange(n_edges), np.arange(n_edges)])
    order = np.arange(2 * n_edges)
    perm = np.lexsort((-order, keys))
    k_sorted = keys[perm]
    first = np.ones(len(k_sorted), dtype=bool)
    first[1:] = k_sorted[1:] != k_sorted[:-1]
    win = perm[first]
    ukeys = k_sorted[first]
    rows = (ukeys // n).astype(np.int64)
    cols = (ukeys % n).astype(np.int64)
    weids = eid[win]
    vals = e[:, weids].astype(np.float64)  # (H, nnz)
    nheads = e.shape[0]
    rowsum = np.zeros((nheads, n), dtype=np.float64)
    for hh in range(nheads):
        rowsum[hh] = np.bincount(rows, weights=vals[hh], minlength=n)
    w = (vals / rowsum[:, rows]).astype(np.float32)
    return rows, cols, w


def _pieces_for_ps(ps):
    """Decompose each src-block pair's [j*ps, (j+1)*ps) slot range of a dst
    block into PE col-tile-legal (chunk, offset, width, j) pieces.
    Legal (offset, width): widths are multiples of 32 with offset+width<=128
    and offset in {0,32,64,96}; at offset 32 only width 32, at 64 width<=64,
    at 96 width 32, at 0 any (asserts in bass.matmul tile_position)."""
    wmax = {0: 128, 32: 32, 64: 64, 96: 32}
    cb = (NB * ps) // 128  # chunks per dst block
    pieces = [[] for _ in range(cb)]
    for j in range(NB):
        s, rem = j * ps, ps
        while rem:
            c, a = s // 128, s % 128
            wd = min(rem, wmax[a], 128 - a)
            pieces[c].append((a, wd, j, s))
            s += wd
            rem -= wd
    assert sum(len(p) for p in pieces) <= 2 * NB + cb
    return pieces, cb


def _make_tables(e, src, dst, cfg: Cfg):
    """Per-core device tables. Returns (tables, R) where tables is a list
    over cores of dicts with keys gsl, sca (fp8), w4 (fp32)."""
    import ml_dtypes

    n = cfg.n
    rows, cols, w = _entries(e, src, dst, n)
    nheads = w.shape[0]
    bpc = cfg.blocks_per_core

    # global R so the program structure is identical on every core.
    # R is a multiple of 4 => every pair occupies whole 128-slot chunks, so
    # every gather matmul loads a full 128-col stationary (FWL-eligible, no
    # column tiling, no PE array mode switches).
    cnt = np.zeros((n // BLK, NB), dtype=np.int64)
    np.add.at(cnt, (rows // BLK, cols // BLK), 1)
    R = 4 * int(np.ceil(cnt.max() / 128))
    ps = 32 * R
    cb = (NB * ps) // 128  # chunks per dst block
    nch = bpc * cb
    nslots = nch * 128

    tables = []
    for k in range(cfg.n_cores):
        sel = (rows >= k * 1024) & (rows < (k + 1) * 1024)
        r_k, c_k, w_k = rows[sel], cols[sel], w[:, sel]
        b_loc = r_k // BLK - k * bpc  # 0..7
        j_blk = c_k // BLK
        key = b_loc * NB + j_blk
        order = np.argsort(key, kind="stable")
        r_k, c_k, w_k, key = r_k[order], c_k[order], w_k[:, order], key[order]
        grp_start = np.searchsorted(key, key)  # first index of each group
        off_in_pair = np.arange(len(key)) - grp_start
        assert off_in_pair.max(initial=0) < ps
        slot = key * ps + off_in_pair  # key = b_loc * NB + j_blk

        gsl = np.zeros((128, nslots), dtype=ml_dtypes.float8_e4m3fn)
        sca = np.zeros((128, nslots), dtype=ml_dtypes.float8_e4m3fn)
        w4 = np.zeros((128, nch, nheads), dtype=np.float32)
        gsl[c_k % BLK, slot] = 1.0
        sca[slot % 128, (slot // 128) * 128 + (r_k % BLK)] = 1.0
        w4[slot % 128, slot // 128, :] = w_k.T
        tables.append(
            {
                "gsl": gsl,
                "sca": sca,
                "w4": np.ascontiguousarray(w4.reshape(128, nch * nheads)),
                "w4h": np.ascontiguousarray(
                    w4.reshape(128, nch * nheads).astype(np.float16)
                ),
            }
        )
    return tables, R


# ------------------------------------------------------------ bass program

_FP32 = mybir.dt.float32
_FP16 = mybir.dt.float16
_FP8 = mybir.dt.float8e4


def _build_program(cfg: Cfg, R: int):
    n = cfg.n
    bpc = cfg.blocks_per_core
    rpc = cfg.rows_per_core
    ps = 32 * R
    pieces, cb = _pieces_for_ps(ps)
    nch = bpc * cb
    nslots = nch * 128
    assert cb % 8 == 0
    nwaves = cb // 8

    WCH = 16  # chunks per wave (one wave = 2 PSUM banks of gathered rows)
    assert cb % WCH == 0
    waves_pb = cb // WCH
    total_waves = bpc * waves_pb

    nc = bacc.Bacc(
        "TRN2",
        target_bir_lowering=False,
        debug=False,
        num_devices=cfg.n_cores,
    )

    # partition-major x layouts: [core, p, b, f] so SBUF loads are contiguous
    xin16 = nc.dram_tensor("xin16", [128, NB * D], _FP16, kind="ExternalInput").ap()
    x0s_d = nc.dram_tensor("x0s", [rpc, D], _FP32, kind="ExternalInput").ap()
    gsl_d = nc.dram_tensor("gsl", [128, nslots], _FP8, kind="ExternalInput").ap()
    sca_d = nc.dram_tensor("sca", [128, nslots], _FP8, kind="ExternalInput").ap()
    w4_d = nc.dram_tensor("w4", [128, nch * H], _FP32, kind="ExternalInput").ap()
    w4h_d = nc.dram_tensor("w4h", [128, nch * H], _FP16, kind="ExternalInput").ap()
    out_d = nc.dram_tensor("out", [rpc, D], _FP32, kind="ExternalOutput").ap()

    xall = nc.dram_tensor(
        "xall", [cfg.n_cores, 128, bpc * D], _FP16, addr_space="Shared"
    ).ap()
    slice_in = nc.dram_tensor("slice_in", [128, bpc * D], _FP16).ap()
    warm_i = nc.dram_tensor("warm_i", [64], _FP32).ap()
    warm_o = nc.dram_tensor("warm_o", [cfg.n_cores, 64], _FP32, addr_space="Shared").ap()

    groups = [list(range(cfg.n_cores))]

    def gather_wave(gw, xsb, psG):
        b, wv = divmod(gw, waves_pb)
        for ci in range(WCH):
            c = wv * WCH + ci
            for (a, wd, j, s_blk) in pieces[c]:
                gofs = b * NB * ps + s_blk
                nc.tensor.matmul(
                    psG[a : a + wd, ci, :],
                    lhsT=gsl_sb[:, gofs : gofs + wd],
                    rhs=xsb[:, j, :],
                    start=True,
                    stop=True,
                    tile_position=(0, a),
                )

    def scatter_wave(gw, psY, xgw):
        b, wv = divmod(gw, waves_pb)
        for ci in range(WCH):
            c = wv * WCH + ci
            gc = b * cb + c
            nc.tensor.matmul(
                psY[:],
                lhsT=sca_sb[:, gc * 128 : (gc + 1) * 128],
                rhs=xgw[:, ci, :],
                start=(c == 0),
                stop=(c == cb - 1),
            )

    with tile.TileContext(nc) as tc:
        with (
            tc.tile_pool(name="tables", bufs=1) as tp,
            tc.tile_pool(name="xsb", bufs=2) as xsbp,
            tc.tile_pool(name="xg", bufs=3) as xgp,
            tc.tile_pool(name="xgw", bufs=3) as xgwp,
            tc.tile_pool(name="acc", bufs=1) as accp,
            tc.tile_pool(name="stage", bufs=2) as stp,
            tc.tile_pool(name="psg", bufs=3, space="PSUM") as ppg,
            tc.tile_pool(name="psy", bufs=2, space="PSUM") as ppy,
        ):
            # warmup collective: aligns the cores' CC streams while tables load
            nc.gpsimd.collective_compute(
                "AllGather",
                mybir.AluOpType.bypass,
                replica_groups=groups,
                ins=[warm_i],
                outs=[warm_o],
            )

            gsl_sb = tp.tile([128, nslots], _FP8)
            sca_sb = tp.tile([128, nslots], _FP8)
            w4_sb = tp.tile([128, nch, H], _FP32)
            w4h_sb = tp.tile([128, nch, H], _FP16)
            nc.sync.dma_start(out=gsl_sb[:], in_=gsl_d)
            nc.sync.dma_start(out=sca_sb[:], in_=sca_d)
            nc.sync.dma_start(out=w4_sb[:].rearrange("p c h -> p (c h)"), in_=w4_d)
            nc.sync.dma_start(out=w4h_sb[:].rearrange("p c h -> p (c h)"), in_=w4h_d)

            # Taylor accumulator starts at the identity term (this core's x0).
            result = accp.tile([128, bpc, D], _FP32)
            nc.sync.dma_start(
                out=result[:],
                in_=x0s_d.rearrange("(j p) f -> p j f", p=128),
            )

            for it in range(1, K_TAYLOR + 1):
                coef = 1.0 / math.factorial(it)
                xsb = xsbp.tile([128, NB, D], _FP16, tag="xsb")
                if it == 1:
                    for g in range(4):
                        nc.sync.dma_start(
                            out=xsb[:, g * 16 : (g + 1) * 16, :],
                            in_=xin16.rearrange("p (b f) -> p b f", f=D)[
                                :, g * 16 : (g + 1) * 16, :
                            ],
                        )
                else:
                    src_ap = xall.rearrange("k p (b f) -> p k b f", f=D)
                    dst_ap = xsb[:].rearrange("p (k b) f -> p k b f", b=bpc)
                    for g in range(4):
                        nc.sync.dma_start(
                            out=dst_ap[:, g * 2 : (g + 1) * 2, :, :],
                            in_=src_ap[:, g * 2 : (g + 1) * 2, :, :],
                        )

                xnext = stp.tile([128, bpc, D], _FP16, tag="xnext")
                psY = None
                tiles = {}

                def weight_mul(gw):
                    b, wv = divmod(gw, waves_pb)
                    psG, _ = tiles[gw]
                    xgw = xgwp.tile([128, WCH, D], _FP16, tag="xgw")
                    tiles[gw] = (psG, xgw)
                    gc0 = b * cb + wv * WCH
                    xgw4 = xgw[:].rearrange("p c (h f) -> p c h f", h=H)
                    if gw % 2 == 0:
                        # DVE reads PSUM directly (fp32, 1x mode)
                        psG4 = psG[:].rearrange("p c (h f) -> p c h f", h=H)
                        w4v = (
                            w4_sb[:, gc0 : gc0 + WCH, :]
                            .unsqueeze(3)
                            .to_broadcast([128, WCH, H, d])
                        )
                        nc.vector.tensor_mul(xgw4, psG4, w4v)
                    else:
                        # ACT evacuates PSUM -> fp16, DVE multiplies in 2x mode
                        xg = xgp.tile([128, WCH, D], _FP16, tag="xg")
                        nc.scalar.copy(xg[:], psG[:])
                        xg4 = xg[:].rearrange("p c (h f) -> p c h f", h=H)
                        w4v = (
                            w4h_sb[:, gc0 : gc0 + WCH, :]
                            .unsqueeze(3)
                            .to_broadcast([128, WCH, H, d])
                        )
                        nc.vector.tensor_mul(xgw4, xg4, w4v)

                # software-pipelined wave loop: gathers run one wave ahead of
                # scatters so the PE never stalls on the weight multiply
                psY_pending = []
                for gw in range(total_waves + 1):
                    if gw < total_waves:
                        if gw % waves_pb == 0:
                            psY_pending.append(
                                ppy.tile([128, D], _FP32, name="psY", tag="psY")
                            )
                        psG = ppg.tile([128, WCH, D], _FP32, tag="psG")
                        tiles[gw] = (psG, None)
                        gather_wave(gw, xsb, psG)
                        weight_mul(gw)
                    if gw >= 1:
                        pgw = gw - 1
                        b, wv = divmod(pgw, waves_pb)
                        if wv == 0:
                            psY = psY_pending.pop(0)
                        scatter_wave(pgw, psY, tiles.pop(pgw)[1])
                        if wv == waves_pb - 1:
                            nc.scalar.copy(xnext[:, b, :], psY[:])
                            nc.vector.scalar_tensor_tensor(
                                result[:, b, :],
                                psY[:],
                                coef,
                                result[:, b, :],
                                op0=mybir.AluOpType.mult,
                                op1=mybir.AluOpType.add,
                            )

                if it < K_TAYLOR:
                    nc.sync.dma_start(
                        out=slice_in, in_=xnext[:].rearrange("p b f -> p (b f)")
                    )
                    nc.gpsimd.collective_compute(
                        "AllGather",
                        mybir.AluOpType.bypass,
                        replica_groups=groups,
                        ins=[slice_in],
                        outs=[xall],
                    )

            nc.sync.dma_start(
                out=out_d.rearrange("(j p) f -> p j f", p=128),
                in_=result[:],
            )

    nc.compile()
    return nc


# ------------------------------------------------------------------ driver

_CACHE = {}


def _get_program(cfg: Cfg, R: int):
    key = (cfg, R)
    if key not in _CACHE:
        _CACHE[key] = _build_program(cfg, R)
    return _CACHE[key]


def _in_maps(x0, tables, cfg: Cfg):
    rpc = cfg.rows_per_core
    # partition-major: xin16[p, B*D + f] = x0[B*128 + p, f]
    x16 = np.ascontiguousarray(
        x0.astype(np.float16).reshape(NB, 128, D).transpose(1, 0, 2).reshape(128, NB * D)
    )
    return [
        {
            "xin16": x16,
            "x0s": np.ascontiguousarray(x0[k * rpc : (k + 1) * rpc]),
            "gsl": t["gsl"],
            "sca": t["sca"],
            "w4": t["w4"],
            "w4h": t["w4h"],
        }
        for k, t in enumerate(tables)
    ]


def run(h, e, src, dst, cfg: Cfg = Cfg(), trace: bool = False):
    """Full pipeline: preprocess, build/compile (cached), execute, assemble."""
    h = np.asarray(h, dtype=np.float32)
    e = np.asarray(e, dtype=np.float32)
    src = np.asarray(src)
    dst = np.asarray(dst)
    nheads = e.shape[0]
    n = h.shape[0]
    dd = h.shape[1] // nheads
    assert (n, nheads, dd) == (cfg.n, H, d), (n, nheads, dd)

    tables, R = _make_tables(e, src, dst, cfg)
    x0 = np.ascontiguousarray(
        h.reshape(nheads, n, dd).transpose(1, 0, 2).reshape(n, nheads * dd)
    )
    nc = _get_program(cfg, R)
    res = run_bass_kernel_spmd(
        nc,
        _in_maps(x0, tables, cfg),
        list(range(cfg.n_cores)),
        trace=trace,
    )
    out = np.concatenate(
        [res.results[k]["out"] for k in range(cfg.n_cores)], axis=0
    )
    # back to reference layout: (n, H, d) node-major -> (H, n, d) -> (N, D)
    out = np.ascontiguousarray(out.reshape(n, nheads, dd).transpose(1, 0, 2)).reshape(
        n, nheads * dd
    )
    return out, res


def kernel(h, e, src, dst):
    out, _ = run(h, e, src, dst)
    return out
